# revision 1
# baseline (speedup 1.0000x reference)
"""TransformerConv MixerBlock (x + TransformerConv(x, edge_index)) on 8 trn2 NeuronCores.

Strategy: permute+bin-pack nodes into 128-node tiles balanced by in-degree
(49 tiles/core). Each core builds the full K/V table (fp16, 512B rows) from x,
then processes its own destination tiles: one indirect-DMA gather of the
incident edges' kv rows per tile, one-hot matmuls to recover q per edge and to
scatter-accumulate exp(alpha)*[v|1] into PSUM, then normalize + skip + residual.
"""
import sys, os, types, math, heapq
sys.path.insert(0, '/opt/trn_rl_repo')
import numpy as np

P = 128
D = 128
H = 4
DH = 32
NCORES = 8

_prog_cache = {}


def _ensure_hooks():
    """Best-effort shim of antenv.axon_hooks so trace=True profiling works."""
    try:
        import antenv
        if 'antenv.axon_hooks' not in sys.modules:
            mod = types.ModuleType('antenv.axon_hooks')
            state = {'hook': None}
            mod.set_axon_ntff_profile_hook = lambda h: state.__setitem__('hook', h)
            mod.get_axon_ntff_profile_hook = lambda: state['hook']
            sys.modules['antenv.axon_hooks'] = mod
            antenv.axon_hooks = mod
            from trn_agent_boot.trn_boot import _ntff_profile_via_ctypes
            hook = _ntff_profile_via_ctypes('/opt/axon/libaxon_pjrt.so')
            if hook is not None:
                mod.set_axon_ntff_profile_hook(hook)
    except Exception:
        pass
    try:
        import concourse.bass_utils as bass_utils
        bass_utils.upload_artifacts = lambda tmpdir: tmpdir
    except Exception:
        pass


def _prep(x, edge_index, Wq, bq, Wk, bk, Wv, bv, Wskip, bskip):
    N = x.shape[0]
    E = edge_index.shape[1]
    TPC = (N + NCORES * P - 1) // (NCORES * P)
    NT = NCORES * TPC

    src = np.asarray(edge_index[0], dtype=np.int64)
    dst = np.asarray(edge_index[1], dtype=np.int64)
    deg = np.bincount(dst, minlength=N)

    # --- bin-pack nodes into NT tiles of <=P nodes, balancing degree sums ---
    order = np.argsort(-deg, kind='stable')
    heap = [(0, t) for t in range(NT)]
    heapq.heapify(heap)
    counts = np.zeros(NT, dtype=np.int64)
    tile_deg = np.zeros(NT, dtype=np.int64)
    node_slot = np.empty(N, dtype=np.int64)
    for n in order:
        while True:
            dsum, t = heapq.heappop(heap)
            if counts[t] < P:
                break
        node_slot[n] = t * P + counts[t]
        counts[t] += 1
        tile_deg[t] += deg[n]
        if counts[t] < P:
            heapq.heappush(heap, (dsum + int(deg[n]), t))
    K = max(1, int((tile_deg.max() + P - 1) // P))

    # --- permuted node features ---
    x_perm = np.zeros((NT * P, D), dtype=np.float16)
    x_perm[node_slot] = np.asarray(x, dtype=np.float16)

    # --- per-tile edge lists sorted by src slot, padded to K*P ---
    src_slot = node_slot[src]
    dst_slot = node_slot[dst]
    et = dst_slot // P            # destination tile of each edge
    key = et * (1 << 32) + src_slot
    eorder = np.argsort(key, kind='stable')
    et_s = et[eorder]
    src_s = src_slot[eorder].astype(np.int32)
    dloc_s = (dst_slot[eorder] - et_s * P).astype(np.int64)

    ecnt = np.bincount(et, minlength=NT)
    eoff = np.zeros(NT + 1, dtype=np.int64)
    np.cumsum(ecnt, out=eoff[1:])
    pos = np.arange(E) - eoff[et_s]          # position within its tile
    flat = et_s * (K * P) + pos

    src_pad = np.zeros(NT * K * P, dtype=np.int32)
    dloc_pad = np.full(NT * K * P, 255, dtype=np.int64)
    src_pad[flat] = src_s
    dloc_pad[flat] = dloc_s

    # gather layout [NT, P, K]: slot (j*P + p) -> [p, j]
    src3 = src_pad.reshape(NT, K, P)
    # chunk j of any tile only references kv rows < hb[j] (edges sorted by src)
    hb = src3.max(axis=(0, 2)).astype(np.int64) + 1
    NBP = 4 * P
    hb = np.minimum(((hb + NBP - 1) // NBP) * NBP, NT * P)
    hb = np.maximum.accumulate(hb)
    src_g = src3.transpose(0, 2, 1).copy()
    dloc_g = dloc_pad.reshape(NT, K, P).transpose(0, 2, 1).astype(np.float32).copy()
    # one-hot [NT, P(r), K*P(c)] where col c = j*P + e
    oh = (dloc_pad.reshape(NT, 1, K * P) == np.arange(P).reshape(1, P, 1))
    oh = oh.astype(np.float16)

    s = 1.0 / math.sqrt(DH)
    wkT = np.asarray(Wk, dtype=np.float32).T.astype(np.float16).copy()
    wvT = np.asarray(Wv, dtype=np.float32).T.astype(np.float16).copy()
    wqT = (np.asarray(Wq, dtype=np.float32).T * s).astype(np.float16).copy()
    wsT = np.asarray(Wskip, dtype=np.float32).T.astype(np.float16).copy()
    for b in (bq, bk, bv, bskip):
        assert np.abs(np.asarray(b)).max() == 0.0, "nonzero biases not supported"
    iota = np.tile(np.arange(P, dtype=np.float16).reshape(1, P), (P, 1)).copy()

    in_maps = []
    for c in range(NCORES):
        t0, t1 = c * TPC, (c + 1) * TPC
        in_maps.append({
            "x_perm": x_perm,
            "x_loc": x_perm[t0 * P:t1 * P].copy(),
            "wkT": wkT, "wvT": wvT, "wqT": wqT, "wsT": wsT, "iota": iota,
            "src_idx": src_g[t0:t1].reshape(TPC * P, K).copy(),
            "dloc": dloc_g[t0:t1].reshape(TPC * P, K).copy(),
            "oh": oh[t0:t1].reshape(TPC * P, K * P).copy(),
        })
    return dict(N=N, E=E, TPC=TPC, NT=NT, K=K, node_slot=node_slot,
                hb=tuple(int(v) for v in hb), in_maps=in_maps)


def _build(TPC, NT, K, HB):
    import concourse.bass as bass
    import concourse.bacc as bacc
    import concourse.mybir as mybir
    import concourse.tile as tile

    f16 = mybir.dt.float16
    f32 = mybir.dt.float32
    i32 = mybir.dt.int32
    MUL = mybir.AluOpType.mult
    ADD = mybir.AluOpType.add
    ISEQ = mybir.AluOpType.is_equal
    EXP = mybir.ActivationFunctionType.Exp
    COPY = mybir.ActivationFunctionType.Copy

    nc = bacc.Bacc("TRN2", target_bir_lowering=False, debug=False)
    x_perm = nc.dram_tensor("x_perm", [NT * P, D], f16, kind="ExternalInput")
    x_loc = nc.dram_tensor("x_loc", [TPC * P, D], f16, kind="ExternalInput")
    wkT = nc.dram_tensor("wkT", [D, D], f16, kind="ExternalInput")
    wvT = nc.dram_tensor("wvT", [D, D], f16, kind="ExternalInput")
    wqT = nc.dram_tensor("wqT", [D, D], f16, kind="ExternalInput")
    wsT = nc.dram_tensor("wsT", [D, D], f16, kind="ExternalInput")
    iota = nc.dram_tensor("iota", [P, P], f16, kind="ExternalInput")
    src_idx = nc.dram_tensor("src_idx", [TPC * P, K], i32, kind="ExternalInput")
    dloc = nc.dram_tensor("dloc", [TPC * P, K], f32, kind="ExternalInput")
    oh_in = nc.dram_tensor("oh", [TPC * P, K * P], f16, kind="ExternalInput")
    out_t = nc.dram_tensor("out", [TPC * P, D], f32, kind="ExternalOutput")

    kv_table = nc.dram_tensor("kv_table", [NT * P, 256], f16)

    NB = 4
    assert NT % NB == 0
    groups = [(g * 8, min(8, K - g * 8)) for g in range((K + 7) // 8)]

    with tile.TileContext(nc) as tc:
        with (
            tc.tile_pool(name="const", bufs=1) as cp,
            tc.tile_pool(name="sbuf", bufs=4) as sb,
            tc.tile_pool(name="big", bufs=4) as bigp,
            tc.tile_pool(name="psA", bufs=2, space="PSUM") as psA,
            tc.tile_pool(name="psB", bufs=2, space="PSUM") as psB,
        ):
            wkv_sb = cp.tile([D, 256], f16, tag="wkv")
            wqs_sb = cp.tile([D, 256], f16, tag="wqs")
            iota_sb = cp.tile([P, P], f16, tag="iota")
            q_loc = cp.tile([P, TPC * D], f16, tag="qloc")
            s_loc = cp.tile([P, TPC * D], f16, tag="sloc")
            nc.sync.dma_start(out=wkv_sb[:, 0:128], in_=wkT[:])
            nc.sync.dma_start(out=wkv_sb[:, 128:256], in_=wvT[:])
            nc.sync.dma_start(out=wqs_sb[:, 0:128], in_=wqT[:])
            nc.sync.dma_start(out=wqs_sb[:, 128:256], in_=wsT[:])
            nc.sync.dma_start(out=iota_sb[:], in_=iota[:])

            # ---------------- node phase: full kv table ----------------
            for it in range(NT // NB):
                t0 = it * NB
                xT = sb.tile([P, NB * P], f16, tag="xT")
                nc.sync.dma_start(
                    out=xT[:], in_=x_perm[t0 * P:(t0 + NB) * P, :], transpose=True)
                pkv = psA.tile([P, NB * 256], f32, tag="pbig")
                for b in range(NB):
                    nc.tensor.matmul(pkv[:, b * 256:(b + 1) * 256],
                                     lhsT=xT[:, b * P:(b + 1) * P], rhs=wkv_sb[:],
                                     start=True, stop=True)
                kvt = sb.tile([P, NB * 256], f16, tag="kvt")
                nc.scalar.activation(out=kvt[:], in_=pkv[:], func=COPY)
                nc.sync.dma_start(
                    out=kv_table[t0 * P:(t0 + NB) * P, :].rearrange(
                        "(b p) c -> p b c", p=P),
                    in_=kvt[:].rearrange("p (b c) -> p b c", c=256))

            # ---------------- local phase: q and skip ----------------
            u = 0
            while u < TPC:
                lb = min(NB, TPC - u)
                xTl = sb.tile([P, NB * P], f16, tag="xT")
                nc.sync.dma_start(
                    out=xTl[:, :lb * P], in_=x_loc[u * P:(u + lb) * P, :],
                    transpose=True)
                pq = psA.tile([P, NB * 256], f32, tag="pbig")
                for b in range(lb):
                    nc.tensor.matmul(pq[:, b * 256:(b + 1) * 256],
                                     lhsT=xTl[:, b * P:(b + 1) * P], rhs=wqs_sb[:],
                                     start=True, stop=True)
                nc.scalar.activation(
                    out=q_loc[:, u * D:(u + lb) * D].rearrange(
                        "p (b c) -> p b c", c=P),
                    in_=pq[:, :lb * 256].rearrange(
                        "p (b c) -> p b c", c=256)[:, :, 0:128], func=COPY)
                xl = sb.tile([P, NB, P], f16, tag="xl")
                nc.sync.dma_start(
                    out=xl[:, :lb, :],
                    in_=x_loc[u * P:(u + lb) * P, :].rearrange(
                        "(b p) c -> p b c", p=P))
                nc.vector.tensor_tensor(
                    out=s_loc[:, u * D:(u + lb) * D].rearrange(
                        "p (b c) -> p b c", c=P),
                    in0=pq[:, :lb * 256].rearrange(
                        "p (b c) -> p b c", c=256)[:, :, 128:256],
                    in1=xl[:, :lb, :], op=ADD)
                u += lb

            # ---------------- edge phase ----------------
            for u in range(TPC):
                idx = sb.tile([P, K], i32, tag="idx")
                nc.sync.dma_start(out=idx[:], in_=src_idx[u * P:(u + 1) * P, :])
                dl = sb.tile([P, K], f32, tag="dl")
                nc.sync.dma_start(out=dl[:], in_=dloc[u * P:(u + 1) * P, :])
                kv_g = bigp.tile([P, K, 256], f16, tag="kvg")
                for j in range(K):
                    nc.gpsimd.indirect_dma_start(
                        out=kv_g[:, j, :], out_offset=None,
                        in_=kv_table[0:HB[j], :],
                        in_offset=bass.IndirectOffsetOnAxis(
                            ap=idx[:, j:j + 1], axis=0))
                oh = bigp.tile([P, K * P], f16, tag="oh")
                nc.sync.dma_start(out=oh[:], in_=oh_in[u * P:(u + 1) * P, :])

                psS = psB.tile([P, 132], f32, tag="acc")
                for (j0, gsz) in groups:
                    pqe = psA.tile([P, NB * 256], f32, tag="pbig")
                    for jj in range(gsz):
                        j = j0 + jj
                        nc.tensor.matmul(
                            pqe[:, jj * P:(jj + 1) * P],
                            lhsT=oh[:, j * P:(j + 1) * P],
                            rhs=q_loc[:, u * D:(u + 1) * D],
                            start=True, stop=True)
                    ohT = sb.tile([P, 8, P], f16, tag="ohT")
                    for jj in range(gsz):
                        j = j0 + jj
                        nc.vector.tensor_scalar(
                            out=ohT[:, jj, :], in0=iota_sb[:],
                            scalar1=dl[:, j:j + 1], scalar2=None, op0=ISEQ)
                    qk = sb.tile([P, 8, P], f16, tag="qk")
                    nc.vector.tensor_tensor(
                        out=qk[:, :gsz, :],
                        in0=pqe[:, :gsz * P].rearrange("p (a c) -> p a c", c=P),
                        in1=kv_g[:, j0:j0 + gsz, 0:128], op=MUL)
                    # reduce 32 -> 1 within each head, via add tree (2x mode)
                    t16 = sb.tile([P, 8 * H, 16], f16, tag="t16")
                    qkv = qk[:, :gsz, :].rearrange("p a (h e) -> p (a h) e", e=DH)
                    nc.vector.tensor_tensor(out=t16[:, :gsz * H, :],
                                            in0=qkv[:, :, 0:16],
                                            in1=qkv[:, :, 16:32], op=ADD)
                    t8 = sb.tile([P, 8 * H, 8], f16, tag="t8")
                    nc.vector.tensor_tensor(out=t8[:, :gsz * H, :],
                                            in0=t16[:, :gsz * H, 0:8],
                                            in1=t16[:, :gsz * H, 8:16], op=ADD)
                    t4 = sb.tile([P, 8 * H, 4], f16, tag="t4")
                    nc.vector.tensor_tensor(out=t4[:, :gsz * H, :],
                                            in0=t8[:, :gsz * H, 0:4],
                                            in1=t8[:, :gsz * H, 4:8], op=ADD)
                    t2 = sb.tile([P, 8 * H, 2], f16, tag="t2")
                    nc.vector.tensor_tensor(out=t2[:, :gsz * H, :],
                                            in0=t4[:, :gsz * H, 0:2],
                                            in1=t4[:, :gsz * H, 2:4], op=ADD)
                    alpha = sb.tile([P, 8 * H, 1], f16, tag="alpha")
                    nc.vector.tensor_tensor(out=alpha[:, :gsz * H, :],
                                            in0=t2[:, :gsz * H, 0:1],
                                            in1=t2[:, :gsz * H, 1:2], op=ADD)
                    X = sb.tile([P, 8, 132], f16, tag="X")
                    nc.scalar.activation(
                        out=X[:, :gsz, 128:132],
                        in_=alpha[:, :gsz * H, 0:1].rearrange(
                            "p (a h) e -> p a (h e)", h=H),
                        func=EXP)
                    nc.vector.tensor_tensor(
                        out=X[:, :gsz, 0:128].rearrange("p a (h e) -> p a h e", e=DH),
                        in0=kv_g[:, j0:j0 + gsz, 128:256].rearrange(
                            "p a (h e) -> p a h e", e=DH),
                        in1=X[:, :gsz, 128:132, None].to_broadcast([P, gsz, H, DH]),
                        op=MUL)
                    for jj in range(gsz):
                        j = j0 + jj
                        nc.tensor.matmul(
                            psS[:, 0:132], lhsT=ohT[:, jj, :], rhs=X[:, jj, 0:132],
                            start=(j == 0), stop=(j == K - 1))
                dn = sb.tile([P, H], f32, tag="dn")
                nc.vector.tensor_scalar(out=dn[:], in0=psS[:, 128:132],
                                        scalar1=1e-16, scalar2=None, op0=ADD)
                rc = sb.tile([P, H], f32, tag="rc")
                nc.vector.reciprocal(out=rc[:], in_=dn[:])
                ot = sb.tile([P, D], f32, tag="ot")
                nc.vector.tensor_tensor(
                    out=ot[:].rearrange("p (h e) -> p h e", e=DH),
                    in0=psS[:, 0:128].rearrange("p (h e) -> p h e", e=DH),
                    in1=rc[:, :, None].to_broadcast([P, H, DH]), op=MUL)
                of = sb.tile([P, D], f32, tag="of")
                nc.vector.tensor_tensor(
                    out=of[:], in0=ot[:], in1=s_loc[:, u * D:(u + 1) * D], op=ADD)
                nc.sync.dma_start(out=out_t[u * P:(u + 1) * P, :], in_=of[:])

    nc.finalize()
    return nc


def _run(inputs, trace=False):
    _ensure_hooks()
    from concourse.bass_utils import run_bass_kernel_spmd

    meta = _prep(**inputs)
    key = (meta['TPC'], meta['NT'], meta['K'], meta['hb'])
    if key not in _prog_cache:
        _prog_cache[key] = _build(*key)
    nc = _prog_cache[key]
    res = run_bass_kernel_spmd(nc, meta['in_maps'],
                               core_ids=list(range(NCORES)), trace=trace)
    outs = [res.results[c]["out"] for c in range(NCORES)]
    out_perm = np.concatenate(outs, axis=0)
    out = out_perm[meta['node_slot']].astype(np.float32)
    return out, res


def kernel(**inputs) -> np.ndarray:
    out, _ = _run(inputs, trace=False)
    return out



# revision 6
# speedup vs baseline: 1.5734x; 1.5734x over previous
"""TransformerConv MixerBlock (x + TransformerConv(x, edge_index)) on 8 trn2 NeuronCores.

Strategy: permute+bin-pack nodes into 128-node tiles balanced by in-degree
(49 tiles/core). Each core builds the full K/V table (fp16, 512B rows) from x,
then processes its own destination tiles: one batched dma_gather per
(tile, table-half) fetches all incident edges' kv rows, one-hot matmuls recover
q per edge and scatter-accumulate exp(alpha)*[v|1] into PSUM, then
normalize + skip + residual.
"""
import sys, os, types, math, heapq
sys.path.insert(0, '/opt/trn_rl_repo')
import numpy as np

P = 128
D = 128
H = 4
DH = 32
NCORES = 8
RB = 32768          # region boundary: int16 gather indices address < 32768 rows
KVBUFS = 3          # kv_g pool depth; first KVBUFS tiles use idx-0 padding

_prog_cache = {}


def _ensure_hooks():
    """Best-effort shim of antenv.axon_hooks so trace=True profiling works."""
    try:
        import antenv
        if 'antenv.axon_hooks' not in sys.modules:
            mod = types.ModuleType('antenv.axon_hooks')
            state = {'hook': None}
            mod.set_axon_ntff_profile_hook = lambda h: state.__setitem__('hook', h)
            mod.get_axon_ntff_profile_hook = lambda: state['hook']
            sys.modules['antenv.axon_hooks'] = mod
            antenv.axon_hooks = mod
            from trn_agent_boot.trn_boot import _ntff_profile_via_ctypes
            hook = _ntff_profile_via_ctypes('/opt/axon/libaxon_pjrt.so')
            if hook is not None:
                mod.set_axon_ntff_profile_hook(hook)
    except Exception:
        pass
    try:
        import concourse.bass_utils as bass_utils
        bass_utils.upload_artifacts = lambda tmpdir: tmpdir
    except Exception:
        pass


def _wrap_idx16(idx, cols):
    """Pack idx (len cols*16... i.e. n<=cols*16) into the dma_gather int16
    layout [128, cols]: position i -> partition i%16, col i//16, replicated
    across the 8 groups of 16 partitions."""
    n = idx.shape[0]
    buf = np.full(cols * 16, -1, dtype=np.int16)
    buf[:n] = idx
    w = buf.reshape(cols, 16).T            # [16, cols]
    return np.tile(w, (8, 1)).copy()       # [128, cols]


def _prep(x, edge_index, Wq, bq, Wk, bk, Wv, bv, Wskip, bskip):
    N = x.shape[0]
    E = edge_index.shape[1]
    TPC = (N + NCORES * P - 1) // (NCORES * P)
    NT = NCORES * TPC

    src = np.asarray(edge_index[0], dtype=np.int64)
    dst = np.asarray(edge_index[1], dtype=np.int64)
    deg = np.bincount(dst, minlength=N)

    # --- bin-pack nodes into NT tiles of <=P nodes, balancing degree sums ---
    order = np.argsort(-deg, kind='stable')
    heap = [(0, t) for t in range(NT)]
    heapq.heapify(heap)
    counts = np.zeros(NT, dtype=np.int64)
    node_slot = np.empty(N, dtype=np.int64)
    for n in order:
        while True:
            dsum, t = heapq.heappop(heap)
            if counts[t] < P:
                break
        node_slot[n] = t * P + counts[t]
        counts[t] += 1
        if counts[t] < P:
            heapq.heappush(heap, (dsum + int(deg[n]), t))

    # --- permuted node features ---
    xf = np.asarray(x, dtype=np.float32)
    x_perm = np.zeros((NT * P, D), dtype=np.float16)
    x_perm[node_slot] = xf.astype(np.float16)
    x_permT = np.ascontiguousarray(x_perm.T)          # [128, NT*P]

    # --- per-tile edge lists sorted by src slot ---
    src_slot = node_slot[src]
    dst_slot = node_slot[dst]
    et = dst_slot // P
    key = et * (1 << 32) + src_slot
    eorder = np.argsort(key, kind='stable')
    et_s = et[eorder]
    src_s = src_slot[eorder]
    dloc_s = dst_slot[eorder] - et_s * P

    # region split: region 1 = src rows < RB, region 2 = the rest
    reg2 = src_s >= RB
    # per (tile, region) counts
    n1 = np.bincount(et_s[~reg2], minlength=NT)
    n2 = np.bincount(et_s[reg2], minlength=NT)
    K1 = max(1, int(-(-n1.max() // P)))
    K2 = max(1, int(-(-n2.max() // P)))
    KT = K1 + K2

    # per-tile idx16 blocks (idx-0 padded so every slot is gathered), dloc map
    idx16 = np.empty((NT, 128, KT * 8), dtype=np.int16)
    dloc_pad = np.full((NT, KT * P), 255, dtype=np.int64)
    eoff = np.zeros(NT + 1, dtype=np.int64)
    np.cumsum(n1 + n2, out=eoff[1:])
    for t in range(NT):
        lo, hi = eoff[t], eoff[t + 1]
        s_t = src_s[lo:hi]
        d_t = dloc_s[lo:hi]
        r2 = s_t >= RB
        s1, d1 = s_t[~r2], d_t[~r2]
        s2, d2 = s_t[r2] - RB, d_t[r2]
        i1 = np.zeros(K1 * P, dtype=np.int64); i1[:len(s1)] = s1
        i2 = np.zeros(K2 * P, dtype=np.int64); i2[:len(s2)] = s2
        idx16[t, :, :K1 * 8] = _wrap_idx16(i1, K1 * 8)
        idx16[t, :, K1 * 8:] = _wrap_idx16(i2, K2 * 8)
        dloc_pad[t, :len(d1)] = d1
        dloc_pad[t, K1 * P:K1 * P + len(d2)] = d2

    # dl: [NT, P, KT] fp16, position (col j, part p) = slot j*P+p
    dl = dloc_pad.reshape(NT, KT, P).transpose(0, 2, 1).astype(np.float16).copy()
    # one-hot [NT, P(r), KT*P(c)] where col c = j*P + e
    oh = (dloc_pad.reshape(NT, 1, KT * P) == np.arange(P).reshape(1, P, 1))
    oh = oh.astype(np.float16)

    s = 1.0 / math.sqrt(DH)
    wkT = np.asarray(Wk, dtype=np.float32).T.astype(np.float16).copy()
    wvT = np.asarray(Wv, dtype=np.float32).T.astype(np.float16).copy()
    wqT = (np.asarray(Wq, dtype=np.float32).T * s).astype(np.float16).copy()
    wsT = np.asarray(Wskip, dtype=np.float32).T.astype(np.float16).copy()
    for b in (bq, bk, bv, bskip):
        assert np.abs(np.asarray(b)).max() == 0.0, "nonzero biases not supported"
    iota_bc = np.tile(np.arange(P, dtype=np.float16).reshape(1, 1, P),
                      (P, 8, 1)).reshape(P, 8 * P).copy()

    in_maps = []
    for c in range(NCORES):
        t0, t1 = c * TPC, (c + 1) * TPC
        in_maps.append({
            "x_permT": x_permT,
            "x_locT": np.ascontiguousarray(x_permT[:, t0 * P:t1 * P]),
            "x_loc": x_perm[t0 * P:t1 * P].copy(),
            "wkvT": np.concatenate([wkT, wvT], axis=1).copy(),
            "wqsT": np.concatenate([wqT, wsT], axis=1).copy(),
            "iota_bc": iota_bc,
            "idx16": idx16[t0:t1].reshape(TPC * 128, KT * 8).copy(),
            "dl": dl[t0:t1].reshape(TPC * P, KT).copy(),
            "oh": oh[t0:t1].reshape(TPC * P, KT * P).copy(),
        })
    return dict(N=N, E=E, TPC=TPC, NT=NT, K1=K1, K2=K2,
                node_slot=node_slot, in_maps=in_maps)


def _build(TPC, NT, K1, K2):
    RBT = RB
    import concourse.bass as bass
    import concourse.bacc as bacc
    import concourse.mybir as mybir
    import concourse.tile as tile

    f16 = mybir.dt.float16
    f32 = mybir.dt.float32
    i16 = mybir.dt.int16
    MUL = mybir.AluOpType.mult
    ADD = mybir.AluOpType.add
    ISEQ = mybir.AluOpType.is_equal
    EXP = mybir.ActivationFunctionType.Exp
    COPY = mybir.ActivationFunctionType.Copy
    AXX = mybir.AxisListType.X

    KT = K1 + K2
    nc = bacc.Bacc("TRN2", target_bir_lowering=False, debug=False)
    x_permT = nc.dram_tensor("x_permT", [D, NT * P], f16, kind="ExternalInput")
    x_locT = nc.dram_tensor("x_locT", [D, TPC * P], f16, kind="ExternalInput")
    x_loc = nc.dram_tensor("x_loc", [TPC * P, D], f16, kind="ExternalInput")
    wkvT = nc.dram_tensor("wkvT", [D, 256], f16, kind="ExternalInput")
    wqsT = nc.dram_tensor("wqsT", [D, 256], f16, kind="ExternalInput")
    iota_in = nc.dram_tensor("iota_bc", [P, 8 * P], f16, kind="ExternalInput")
    idx_in = nc.dram_tensor("idx16", [TPC * 128, KT * 8], i16, kind="ExternalInput")
    dl_in = nc.dram_tensor("dl", [TPC * P, KT], f16, kind="ExternalInput")
    oh_in = nc.dram_tensor("oh", [TPC * P, KT * P], f16, kind="ExternalInput")
    out_t = nc.dram_tensor("out", [TPC * P, D], f32, kind="ExternalOutput")

    kv_tab1 = nc.dram_tensor("kv_tab1", [RBT, 256], f16)
    kv_tab2 = nc.dram_tensor("kv_tab2", [NT * P - RBT, 256], f16)

    NB = 4
    assert NT % NB == 0
    groups = [(g * 8, min(8, KT - g * 8)) for g in range((KT + 7) // 8)]

    with tile.TileContext(nc) as tc:
        with (
            tc.tile_pool(name="const", bufs=1) as cp,
            tc.tile_pool(name="sbuf", bufs=4) as sb,
            tc.tile_pool(name="ohp", bufs=3) as ohp,
            tc.tile_pool(name="kvp", bufs=KVBUFS) as kvp,
            tc.tile_pool(name="psA", bufs=3, space="PSUM") as psA,
            tc.tile_pool(name="psB", bufs=2, space="PSUM") as psB,
        ):
            wkv_sb = cp.tile([D, 256], f16, tag="wkv")
            wqs_sb = cp.tile([D, 256], f16, tag="wqs")
            iota_sb = cp.tile([P, 8 * P], f16, tag="iota")
            q_loc = cp.tile([P, TPC * D], f16, tag="qloc")
            s_loc = cp.tile([P, TPC * D], f16, tag="sloc")
            nc.sync.dma_start(out=wkv_sb[:], in_=wkvT[:])
            nc.sync.dma_start(out=wqs_sb[:], in_=wqsT[:])
            nc.sync.dma_start(out=iota_sb[:], in_=iota_in[:])

            # ---------------- local phase: q and skip ----------------
            u = 0
            while u < TPC:
                lb = min(NB, TPC - u)
                xTl = sb.tile([P, NB * P], f16, tag="xT")
                nc.sync.dma_start(out=xTl[:, :lb * P],
                                  in_=x_locT[:, u * P:(u + lb) * P])
                pq = psA.tile([P, NB * 256], f32, tag="pbig")
                for b in range(lb):
                    nc.tensor.matmul(pq[:, b * 256:(b + 1) * 256],
                                     lhsT=xTl[:, b * P:(b + 1) * P], rhs=wqs_sb[:],
                                     start=True, stop=True)
                nc.scalar.activation(
                    out=q_loc[:, u * D:(u + lb) * D].rearrange(
                        "p (b c) -> p b c", c=P),
                    in_=pq[:, :lb * 256].rearrange(
                        "p (b c) -> p b c", c=256)[:, :, 0:128], func=COPY)
                xl = sb.tile([P, NB, P], f16, tag="xl")
                nc.sync.dma_start(
                    out=xl[:, :lb, :],
                    in_=x_loc[u * P:(u + lb) * P, :].rearrange(
                        "(b p) c -> p b c", p=P))
                nc.vector.tensor_tensor(
                    out=s_loc[:, u * D:(u + lb) * D].rearrange(
                        "p (b c) -> p b c", c=P),
                    in0=pq[:, :lb * 256].rearrange(
                        "p (b c) -> p b c", c=256)[:, :, 128:256],
                    in1=xl[:, :lb, :], op=ADD)
                u += lb

            # ---------------- node phase: full kv table ----------------
            for it in range(NT // NB):
                t0 = it * NB
                xT = sb.tile([P, NB * P], f16, tag="xT")
                nc.sync.dma_start(out=xT[:], in_=x_permT[:, t0 * P:(t0 + NB) * P])
                pkv = psA.tile([P, NB * 256], f32, tag="pbig")
                for b in range(NB):
                    nc.tensor.matmul(pkv[:, b * 256:(b + 1) * 256],
                                     lhsT=xT[:, b * P:(b + 1) * P], rhs=wkv_sb[:],
                                     start=True, stop=True)
                kvt = sb.tile([P, NB * 256], f16, tag="kvt")
                nc.scalar.activation(out=kvt[:], in_=pkv[:], func=COPY)
                r0 = t0 * P
                if r0 < RBT:
                    dst = kv_tab1[r0:r0 + NB * P, :]
                else:
                    dst = kv_tab2[r0 - RBT:r0 - RBT + NB * P, :]
                nc.sync.dma_start(
                    out=dst.rearrange("(b p) c -> p b c", p=P),
                    in_=kvt[:].rearrange("p (b c) -> p b c", c=256))

            # ---------------- edge phase ----------------
            for u in range(TPC):
                idx = sb.tile([P, KT * 8], i16, tag="idx")
                nc.sync.dma_start(out=idx[:], in_=idx_in[u * 128:(u + 1) * 128, :])
                dlt = sb.tile([P, KT], f16, tag="dl")
                nc.sync.dma_start(out=dlt[:], in_=dl_in[u * P:(u + 1) * P, :])
                kv_g = kvp.tile([P, KT, 256], f16, tag="kvg")
                # HW limit: <=1024 rows (8 idx columns) per dma_gather
                for (tab, cbase, csz) in (
                        [(kv_tab1, c, min(8, K1 - c)) for c in range(0, K1, 8)]
                        + [(kv_tab2, K1 + c, min(8, K2 - c))
                           for c in range(0, K2, 8)]):
                    nc.gpsimd.dma_gather(
                        kv_g[:, cbase:cbase + csz, :], tab[:],
                        idx[:, cbase * 8:(cbase + csz) * 8],
                        csz * P, csz * P, 256)
                oh = ohp.tile([P, KT * P], f16, tag="oh")
                nc.sync.dma_start(out=oh[:], in_=oh_in[u * P:(u + 1) * P, :])

                psS = psB.tile([P, 132], f32, tag="acc")
                for (j0, gsz) in groups:
                    pqe = psA.tile([P, NB * 256], f32, tag="pbig")
                    for jj in range(gsz):
                        j = j0 + jj
                        nc.tensor.matmul(
                            pqe[:, jj * P:(jj + 1) * P],
                            lhsT=oh[:, j * P:(j + 1) * P],
                            rhs=q_loc[:, u * D:(u + 1) * D],
                            start=True, stop=True)
                    ohT = sb.tile([P, 8, P], f16, tag="ohT")
                    nc.vector.tensor_tensor(
                        out=ohT[:, :gsz, :],
                        in0=iota_sb[:, :gsz * P].rearrange("p (a c) -> p a c", c=P),
                        in1=dlt[:, j0:j0 + gsz, None].to_broadcast([P, gsz, P]),
                        op=ISEQ)
                    qk = sb.tile([P, 8, P], f16, tag="qk")
                    nc.vector.tensor_tensor(
                        out=qk[:, :gsz, :],
                        in0=pqe[:, :gsz * P].rearrange("p (a c) -> p a c", c=P),
                        in1=kv_g[:, j0:j0 + gsz, 0:128], op=MUL)
                    alpha = sb.tile([P, 8 * H], f32, tag="alpha")
                    nc.vector.tensor_reduce(
                        out=alpha[:, :gsz * H],
                        in_=qk[:, :gsz, :].rearrange("p a (h e) -> p (a h) e", e=DH),
                        axis=AXX, op=ADD)
                    X = sb.tile([P, 8, 132], f16, tag="X")
                    nc.scalar.activation(
                        out=X[:, :gsz, 128:132],
                        in_=alpha[:, :gsz * H].rearrange("p (a h) -> p a h", h=H),
                        func=EXP)
                    nc.vector.tensor_tensor(
                        out=X[:, :gsz, 0:128].rearrange("p a (h e) -> p a h e", e=DH),
                        in0=kv_g[:, j0:j0 + gsz, 128:256].rearrange(
                            "p a (h e) -> p a h e", e=DH),
                        in1=X[:, :gsz, 128:132, None].to_broadcast([P, gsz, H, DH]),
                        op=MUL)
                    for jj in range(gsz):
                        j = j0 + jj
                        nc.tensor.matmul(
                            psS[:, 0:132], lhsT=ohT[:, jj, :], rhs=X[:, jj, 0:132],
                            start=(j == 0), stop=(j == KT - 1))
                dn = sb.tile([P, H], f32, tag="dn")
                nc.vector.tensor_scalar(out=dn[:], in0=psS[:, 128:132],
                                        scalar1=1e-16, scalar2=None, op0=ADD)
                rc = sb.tile([P, H], f32, tag="rc")
                nc.vector.reciprocal(out=rc[:], in_=dn[:])
                ot = sb.tile([P, D], f32, tag="ot")
                nc.vector.tensor_tensor(
                    out=ot[:].rearrange("p (h e) -> p h e", e=DH),
                    in0=psS[:, 0:128].rearrange("p (h e) -> p h e", e=DH),
                    in1=rc[:, :, None].to_broadcast([P, H, DH]), op=MUL)
                of = sb.tile([P, D], f32, tag="of")
                nc.vector.tensor_tensor(
                    out=of[:], in0=ot[:], in1=s_loc[:, u * D:(u + 1) * D], op=ADD)
                nc.sync.dma_start(out=out_t[u * P:(u + 1) * P, :], in_=of[:])

    nc.finalize()
    return nc


def _run(inputs, trace=False):
    _ensure_hooks()
    from concourse.bass_utils import run_bass_kernel_spmd

    meta = _prep(**inputs)
    key = (meta['TPC'], meta['NT'], meta['K1'], meta['K2'])
    if key not in _prog_cache:
        _prog_cache[key] = _build(*key)
    nc = _prog_cache[key]
    res = run_bass_kernel_spmd(nc, meta['in_maps'],
                               core_ids=list(range(NCORES)), trace=trace)
    outs = [res.results[c]["out"] for c in range(NCORES)]
    out_perm = np.concatenate(outs, axis=0)
    out = out_perm[meta['node_slot']].astype(np.float32)
    return out, res


def kernel(**inputs) -> np.ndarray:
    out, _ = _run(inputs, trace=False)
    return out
